# revision 1
# baseline (speedup 1.0000x reference)
# Trainium2 Bass kernel for DST_Decoder.
#
# Math reformulation (exact):
#   h  = relu(x @ w1 + b1);  p = h @ w2 + b2                  (pointwise MLP)
#   dx_t = p_t - p_{t-1} (p_{-1}=0);  m_t = (p_t + p_{t-1})/2 = p_t - dx_t/2
#   S1_t = p_t;  S2_t = sum_{s<=t} m_s (x) dx_s               (Chen identity)
#   z_t  = sig_t @ W1 + bb1 = cumsum_t[ vec(m (x) dx) @ W1_sig + dx @ W1_s1 ] + bb1
#   out  = relu(z) @ W2 + bb2
# i.e. contract each timestep's rank-1 outer-product update with W1 FIRST
# (a (T,1056)x(1056,64) matmul), then a cheap 64-wide cumulative scan.
#
# Layout: "transposed land" — features on SBUF partitions, time on the free
# axis.  x is shipped pre-transposed from the host (layout prep is part of
# sharding).  The outer-product tensor O^T (1056, t) is built k-tile by
# k-tile: PE broadcasts rows of m^T to 128 partitions via a 0/1 selection
# matrix (E_r @ m^T -> PSUM), DVE multiplies with a 4x-stacked copy of dx^T.
# Sharding: data-parallel over batch, 4 batches per core, weights replicated.

import os
import sys

import numpy as np

for _p in ("/opt/trn_rl_repo",):
    if _p not in sys.path and os.path.isdir(_p):
        sys.path.append(_p)

from concourse import bacc, tile
from concourse import bass_utils
import concourse.mybir as mybir

F32 = mybir.dt.float32
F32R = mybir.dt.float32r
BF16 = mybir.dt.bfloat16

N_CORES = 8
B, L, DIN = 32, 1024, 256
C, HID, DOUT = 32, 64, 128
B_CORE = B // N_CORES                 # 4 batches per core
T = B_CORE * L                        # 4096 time positions per core
KT = (C * C) // 128                   # 8 k-tiles of the outer-product block

TRACE = False
LAST_EXEC_NS = None
LAST_PROFILE = None
LAST_TRACE_PATH = None


def build_nc(t_total=T, seq_len=L, chunk=512):
    assert t_total % seq_len == 0 and seq_len % chunk == 0 or chunk % seq_len == 0
    nch = t_total // chunk
    n_batch = t_total // seq_len
    ch_per_batch = seq_len // chunk
    assert ch_per_batch * chunk == seq_len

    nc = bacc.Bacc(trn_type="TRN2", target_bir_lowering=False, debug=False)

    xT = nc.dram_tensor("xT", (DIN, t_total), F32R, kind="ExternalInput").ap()
    w1 = nc.dram_tensor("w1", (DIN, HID), F32R, kind="ExternalInput").ap()
    b1c = nc.dram_tensor("b1c", (HID, 1), F32, kind="ExternalInput").ap()
    w2 = nc.dram_tensor("w2", (HID, C), F32R, kind="ExternalInput").ap()
    b2c = nc.dram_tensor("b2c", (C, 1), F32, kind="ExternalInput").ap()
    W1m = nc.dram_tensor("W1m", (KT, 128, HID), F32R, kind="ExternalInput").ap()
    W1dx = nc.dram_tensor("W1dx", (C, HID), F32R, kind="ExternalInput").ap()
    bb1c = nc.dram_tensor("bb1c", (HID, 1), F32, kind="ExternalInput").ap()
    Emat = nc.dram_tensor("Emat", (C, KT * 128), BF16, kind="ExternalInput").ap()
    W2b = nc.dram_tensor("W2b", (HID + 1, DOUT), F32R, kind="ExternalInput").ap()
    ones = nc.dram_tensor("ones", (1, t_total), F32R, kind="ExternalInput").ap()
    out = nc.dram_tensor("out", (DOUT, t_total), F32, kind="ExternalOutput").ap()

    RELU = mybir.ActivationFunctionType.Relu
    ADD = mybir.AluOpType.add
    MUL = mybir.AluOpType.mult
    BYP = mybir.AluOpType.bypass

    with tile.TileContext(nc) as tc:
        with (
            tc.tile_pool(name="consts", bufs=1) as cpool,
            tc.tile_pool(name="persist", bufs=1) as ppool,
            tc.tile_pool(name="xin", bufs=3) as xpool,
            tc.tile_pool(name="hbuf", bufs=2) as hpool,
            tc.tile_pool(name="rhs", bufs=8) as rpool,
            tc.tile_pool(name="zbuf", bufs=2) as zpool,
            tc.tile_pool(name="obuf", bufs=2) as opool,
            tc.tile_pool(name="ps1", bufs=2, space="PSUM") as ps1,
            tc.tile_pool(name="psu", bufs=4, space="PSUM") as psu,
            tc.tile_pool(name="psm", bufs=2, space="PSUM") as psm,
        ):
            # ---- constants -------------------------------------------------
            w1_sb = cpool.tile([128, 2, HID], F32R, tag="w1")
            for k in range(2):
                nc.sync.dma_start(out=w1_sb[:, k, :], in_=w1[k * 128:(k + 1) * 128, :])
            b1_sb = cpool.tile([HID, 1], F32, tag="b1")
            nc.sync.dma_start(out=b1_sb[:], in_=b1c)
            w2_sb = cpool.tile([HID, C], F32R, tag="w2")
            nc.sync.dma_start(out=w2_sb[:], in_=w2)
            b2_sb = cpool.tile([C, 1], F32, tag="b2")
            nc.sync.dma_start(out=b2_sb[:], in_=b2c)
            W1m_sb = cpool.tile([128, KT, HID], F32R, tag="W1m")
            for r in range(KT):
                nc.sync.dma_start(out=W1m_sb[:, r, :], in_=W1m[r])
            W1dx_sb = cpool.tile([C, HID], F32R, tag="W1dx")
            nc.sync.dma_start(out=W1dx_sb[:], in_=W1dx)
            bb1_sb = cpool.tile([HID, 1], F32, tag="bb1")
            nc.sync.dma_start(out=bb1_sb[:], in_=bb1c)
            E_sb = cpool.tile([C, KT * 128], BF16, tag="E")
            nc.sync.dma_start(out=E_sb[:], in_=Emat)
            W2b_sb = cpool.tile([HID + 1, DOUT], F32R, tag="W2b")
            nc.sync.dma_start(out=W2b_sb[:], in_=W2b)

            # ---- persistent activations -----------------------------------
            pT = ppool.tile([C, t_total], F32, tag="pT")
            dxT = ppool.tile([C, t_total], F32, tag="dxT")
            dx4 = ppool.tile([128, t_total], F32, tag="dx4")
            dxr = ppool.tile([C, t_total], F32R, tag="dxr")
            mTr = ppool.tile([C, t_total], BF16, tag="mTr")
            uT = ppool.tile([HID, t_total], F32, tag="uT")
            aT = ppool.tile([HID + 1, t_total], F32R, tag="aT")
            nc.sync.dma_start(out=aT[HID:HID + 1, :], in_=ones)

            # ---- pipeline: batch-pair groups, sectioned weight reuse ------
            IDENT = mybir.ActivationFunctionType.Identity

            def phase1_pair(c0):
                # two chunks with weight-reuse streaks
                chunks = [c0, c0 + 1]
                xts, hpss = [], []
                for c in chunks:
                    cs = slice(c * chunk, (c + 1) * chunk)
                    xt = xpool.tile([128, 2, chunk], F32R, tag="xt")
                    for k in range(2):
                        nc.sync.dma_start(out=xt[:, k, :],
                                          in_=xT[k * 128:(k + 1) * 128, cs])
                    xts.append(xt)
                    hps = ps1.tile([HID, chunk], F32, tag="ps1t")
                    hpss.append(hps)
                for k in range(2):
                    for i in range(2):
                        nc.tensor.matmul(hpss[i][:], w1_sb[:, k, :], xts[i][:, k, :],
                                         start=(k == 0), stop=(k == 1))
                hsbs = []
                for i, c in enumerate(chunks):
                    hsb = hpool.tile([HID, chunk], F32R, tag="hsb")
                    nc.scalar.activation(hsb[:], hpss[i][:], RELU,
                                         bias=b1_sb[:], scale=1.0)
                    hsbs.append(hsb)
                ppss = []
                for i, c in enumerate(chunks):
                    pps = ps1.tile([C, chunk], F32, tag="ps1t")
                    nc.tensor.matmul(pps[:], w2_sb[:], hsbs[i][:],
                                     start=True, stop=True)
                    ppss.append(pps)
                for i, c in enumerate(chunks):
                    cs = slice(c * chunk, (c + 1) * chunk)
                    nc.scalar.add(pT[:, cs], ppss[i][:], b2_sb[:])

            def phase2(b):
                t0 = b * seq_len
                bs = slice(t0, t0 + seq_len)
                nc.gpsimd.tensor_copy(dxT[:, t0:t0 + 1], pT[:, t0:t0 + 1])
                nc.gpsimd.tensor_sub(
                    dxT[:, t0 + 1:t0 + seq_len],
                    pT[:, t0 + 1:t0 + seq_len],
                    pT[:, t0:t0 + seq_len - 1],
                )
                nc.vector.scalar_tensor_tensor(
                    mTr[:, bs], dxT[:, bs], -0.5, pT[:, bs], op0=MUL, op1=ADD,
                )
                nc.scalar.activation(dxr[:, bs], dxT[:, bs], IDENT, bias=0.0, scale=1.0)
                for ii in range(4):
                    nc.sync.dma_start(out=dx4[32 * ii:32 * (ii + 1), bs],
                                      in_=dxT[:, bs])

            def phase3_group(chunks):
                # mb section: one E_r load serves all chunks; DVE muls follow
                rhs_tiles = {}
                for r in range(KT):
                    for c in chunks:
                        cs = slice(c * chunk, (c + 1) * chunk)
                        mb = psm.tile([128, chunk], F32, tag="mb")
                        nc.tensor.matmul(
                            mb[:], E_sb[:, r * 128:(r + 1) * 128], mTr[:, cs],
                            start=True, stop=True,
                        )
                        rhsb = rpool.tile([128, chunk], F32R, tag="rhsb")
                        nc.vector.tensor_mul(rhsb[:], mb[:], dx4[:, cs])
                        rhs_tiles[(r, c)] = rhsb
                # mains section: one W1m_r load serves all chunks
                upss = {}
                for c in chunks:
                    ups = psu.tile([HID, chunk], F32, tag="ups")
                    upss[c] = ups
                for r in range(KT):
                    for c in chunks:
                        nc.tensor.matmul(
                            upss[c][:], W1m_sb[:, r, :], rhs_tiles[(r, c)][:],
                            start=(r == 0), stop=False,
                        )
                for c in chunks:
                    cs = slice(c * chunk, (c + 1) * chunk)
                    nc.tensor.matmul(upss[c][:], W1dx_sb[:], dxr[:, cs],
                                     start=False, stop=True)
                for c in chunks:
                    cs = slice(c * chunk, (c + 1) * chunk)
                    nc.scalar.copy(uT[:, cs], upss[c][:])

            def phase4(b):
                t0 = b * seq_len
                bs = slice(t0, t0 + seq_len)
                zb = zpool.tile([HID, seq_len], F32, tag="zb")
                nc.vector.tensor_tensor_scan(
                    zb[:], uT[:, bs], uT[:, bs], 0.0, op0=ADD, op1=BYP,
                )
                nc.scalar.activation(aT[0:HID, bs], zb[:], RELU,
                                     bias=bb1_sb[:], scale=1.0)

            def phase5(c):
                cs = slice(c * chunk, (c + 1) * chunk)
                ops = ps1.tile([DOUT, chunk], F32, tag="ps1t")
                nc.tensor.matmul(ops[:], W2b_sb[:], aT[:, cs],
                                 start=True, stop=True)
                osb = opool.tile([DOUT, chunk], F32, tag="osb")
                nc.scalar.copy(osb[:], ops[:])
                nc.sync.dma_start(out=out[:, cs], in_=osb[:])

            n_groups = max(1, n_batch // 2)
            bpg = n_batch // n_groups
            cpg = t_total // chunk // n_groups
            for g in range(n_groups):
                for c0 in range(g * cpg, (g + 1) * cpg, 2):
                    phase1_pair(c0)
                for b in range(g * bpg, (g + 1) * bpg):
                    phase2(b)
                phase3_group(list(range(g * cpg, (g + 1) * cpg)))
                for b in range(g * bpg, (g + 1) * bpg):
                    phase4(b)
                for c in range(g * cpg, (g + 1) * cpg):
                    phase5(c)

    nc.compile()
    return nc


def host_prep_shared(w1, b1, w2, b2, W1, bb1, W2, bb2):
    f = np.float32
    E = np.zeros((C, KT * 128), f)
    for r in range(KT):
        for q in range(128):
            E[4 * r + q // 32, 128 * r + q] = 1.0
    return {
        "w1": np.ascontiguousarray(w1, f),
        "b1c": np.ascontiguousarray(b1.reshape(-1, 1), f),
        "w2": np.ascontiguousarray(w2, f),
        "b2c": np.ascontiguousarray(b2.reshape(-1, 1), f),
        "W1m": np.ascontiguousarray(W1[C:].reshape(KT, 128, HID), f),
        "W1dx": np.ascontiguousarray(W1[:C], f),
        "bb1c": np.ascontiguousarray(bb1.reshape(-1, 1), f),
        "Emat": E.astype(__import__("ml_dtypes").bfloat16),
        "W2b": np.ascontiguousarray(np.vstack([W2, bb2[None, :]]), f),
        "ones": np.ones((1, T), f),
    }


_NC_CACHE = {}


def _get_nc():
    key = "full"
    if key not in _NC_CACHE:
        _NC_CACHE[key] = build_nc()
    return _NC_CACHE[key]


def kernel(x, w1, b1, w2, b2, W1, bb1, W2, bb2):
    global LAST_EXEC_NS, LAST_PROFILE, LAST_TRACE_PATH
    x = np.ascontiguousarray(x, np.float32)
    nc = _get_nc()
    shared = host_prep_shared(w1, b1, w2, b2, W1, bb1, W2, bb2)
    in_maps = []
    for core in range(N_CORES):
        xc = x[core * B_CORE:(core + 1) * B_CORE].reshape(T, DIN)
        m = dict(shared)
        m["xT"] = np.ascontiguousarray(xc.T)
        in_maps.append(m)
    try:
        res = bass_utils.run_bass_kernel_spmd(
            nc, in_maps, core_ids=list(range(N_CORES)), trace=TRACE,
        )
    except Exception:
        if not TRACE:
            raise
        res = bass_utils.run_bass_kernel_spmd(
            nc, in_maps, core_ids=list(range(N_CORES)), trace=False,
        )
    LAST_EXEC_NS = res.exec_time_ns
    LAST_PROFILE = res.profile_json
    LAST_TRACE_PATH = (res.instructions_and_trace or (None, None))[1]
    outs = [np.ascontiguousarray(res.results[i]["out"].T).reshape(B_CORE, L, DOUT)
            for i in range(N_CORES)]
    return np.concatenate(outs, axis=0)

